# revision 42
# baseline (speedup 1.0000x reference)
"""Single-head causal attention (B=4, T=4096, E=1024, H=64) on 8 TRN2 NeuronCores.

Sharding: data-parallel over batch (4) x 2-way query-parallel with a snake
block pairing: core c handles batch b=c//2 and 512-query blocks
{j, 3-j, 4+j, 7-j} (j=c%2), 72 causal s-tiles each.  Block m's keys are
exactly chunks 0..m, so block m is fully computable once chunk m is
projected; chunks stream in a per-variant order that spreads flash work
evenly (PE starvation re-throttles the clock via the HAM activity monitor).

Perf-critical details (measured on this silicon):
  * Everything PE-facing is fp16 (x, W, K^T, Q^T): same 2^-11 rounding as
    the tf32 path but half the HBM traffic, 1 cycle/col, and never
    disables fast-weight-load the way fp32 matmuls do.  exp outputs are
    bf16 (fp16 would overflow at e^70; no max-subtraction needed here).
  * ALL matmuls keep a 128-partition contraction dim.  64-partition
    contraction runs at half rate, and alternating 64/128 shapes is ~2.5x
    worse.  The score matmul contracts over [K^T; zeros] stacked to 128
    rows against [Q^T; zeros] (Wq is zero-padded to 128 output cols so the
    projection itself writes the pad rows).
  * A matmul that carries a semaphore wait loses ~173ns of pipelining, so
    work is emitted in batched groups (2 score mm -> 1 wide exp -> 2 PV mm)
    with a lag-2 software pipeline between scores and PV; PV groups may
    complete out of order (PSUM accumulation is order-agnostic).  The
    p-tile pool is 6 deep -- shallower pools stall the PE ~8us.
  * exp runs on ACT over a 2-bank [128,1024] PSUM tile (halves the ~260ns
    per-instruction overhead).
  * Diagonal tiles are trimmed: tile at offset d only has valid queries
    qq >= 128d, so its score/exp/mask/PV run on cols [128d:512] (PSUM
    has_written semantics make sub-range accumulation safe out of order).
  * Two HWDGE rings: weights then x stream FIFO on the Sync ring; masks
    and per-block output stores go on the ACT ring so they never queue
    behind x.  Dependency-free warm-up transposes (more for j=1, whose
    first chunk lands later) hold the HAM busy while the first pieces
    land so real matmuls start at 2.4GHz.
"""

import math
import numpy as np
import ml_dtypes

import concourse.bacc as bacc
import concourse.tile as tile
import concourse.mybir as mybir
from concourse.bass_utils import run_bass_kernel_spmd
from concourse.masks import make_identity

f32 = mybir.dt.float32
f32r = mybir.dt.float32r
f16 = mybir.dt.float16
bf16 = mybir.dt.bfloat16

B, T, E, H = 4, 4096, 1024, 64
NCORES = 8
TCH = 512          # t-chunk = 512-query block
QB = 512
ST = 128           # s (key) tile
N_ETILES = E // 128
N_CH = T // TCH    # 8 chunks
QOWN = T // 2


def _blocks(j):
    return [j, 3 - j, 4 + j, 7 - j]


def build_nc(core_j):
    nc = bacc.Bacc(name=f"attn_j{core_j}")
    xT_d = nc.dram_tensor("xT", [E, T], f16, kind="ExternalInput")
    wkv_d = nc.dram_tensor("wkv", [E, 128], f16, kind="ExternalInput")
    wq_d = nc.dram_tensor("wq", [E, 128], f16, kind="ExternalInput")
    masks_d = nc.dram_tensor("masks", [2, 128, 2 * QB], bf16,
                             kind="ExternalInput")
    out_d = nc.dram_tensor("out", [H + 1, QOWN], bf16,
                           kind="ExternalOutput")

    own = sorted(_blocks(core_j))          # ascending local layout
    n_ch = max(own) + 1                    # j=0: 8 chunks, j=1: 7
    qloc_of = {m: i * QB for i, m in enumerate(own)}
    # chunk streaming order: owned-block queries early so flash work is
    # spread across the stream (avoids PE starvation -> HAM re-throttle)
    ch_order = [0, 3, 1, 2, 4, 7, 5, 6] if core_j == 0 else \
               [1, 0, 2, 3, 5, 6, 4]

    with tile.TileContext(nc) as tc:
        with tc.tile_pool(name="singles", bufs=1) as singles, \
             tc.tile_pool(name="pwork", bufs=6) as pwork, \
             tc.tile_pool(name="psS", bufs=2, space="PSUM") as psS, \
             tc.tile_pool(name="psO", bufs=2, space="PSUM") as psO, \
             tc.tile_pool(name="psKV", bufs=1, space="PSUM") as psKV, \
             tc.tile_pool(name="psQT", bufs=1, space="PSUM") as psQT:

            # ---- weights first on the Sync ring (they gate the first
            # matmul; the ACT ring's queue starts ~2.5us later) ----
            wkv_sb = singles.tile([128, N_ETILES, 128], f16)
            nc.sync.dma_start(out=wkv_sb,
                              in_=wkv_d[:, :].rearrange("(n p) m -> p n m",
                                                        p=128))
            wq_sb = singles.tile([128, N_ETILES, 128], f16)
            mask_sb = singles.tile([128, 2, 2 * QB], bf16)
            nc.scalar.dma_start(out=mask_sb,
                                in_=masks_d[:, :, :].rearrange("m p q -> p m q"))
            identf = singles.tile([128, 128], f32)
            make_identity(nc, identf)
            ident = singles.tile([64, 64], bf16)
            nc.vector.tensor_copy(ident, identf[0:64, 0:64])

            # kvT rows 0:64 = K^T (fp16); rows 64:128 stay ZERO so the score
            # matmul's 128-row contraction adds nothing from the pad side.
            kvT_sb = singles.tile([128, T], f16)
            zscr = singles.tile([64, TCH], f32)
            nc.vector.memset(zscr, 0.0)
            for t in range(0, T, TCH):
                nc.vector.tensor_copy(kvT_sb[64:128, t:t + TCH], zscr)
            qT_sb = singles.tile([128, QOWN], f16)     # [Q^T; zeros]
            vp_sb = singles.tile([128, T // ST, H + 1], bf16)
            nc.vector.memset(vp_sb[:, :, H:H + 1], 1.0)
            oT_sb = singles.tile([H + 1, QOWN], bf16)

            # ---- x stream on the Sync ring in ch_order, e-pair pieces;
            # wq rides after the first chunk (needed by its q projection) ----
            x_sb = singles.tile([128, N_ETILES, T], f16)
            for ci, c in enumerate(ch_order):
                t0 = c * TCH
                for e in range(0, N_ETILES, 2):
                    nc.sync.dma_start(
                        out=x_sb[:, e:e + 2, t0:t0 + TCH],
                        in_=xT_d[e * 128:(e + 2) * 128, t0:t0 + TCH]
                        .rearrange("(n p) m -> p n m", p=128))
                if ci == 0:
                    nc.sync.dma_start(
                        out=wq_sb,
                        in_=wq_d[:, :].rearrange("(n p) m -> p n m", p=128))

            # ---- PE warm-up: ~16 dependency-free transposes keep the HAM
            # activity window busy while the first x pieces land, so real
            # matmuls start at full clock ----
            warm_ps = psKV.tile([128, 128], f32, tag="kv", name="warm")
            for _ in range(16 if core_j == 0 else 30):
                nc.tensor.transpose(warm_ps, identf, identf)

            # ---- flash attention emission (groups of 2 s-tiles) ----
            pend = []          # lag-2 queue of (block, group, p_sb)
            emitted = {m: 0 for m in own}   # PV groups done per block
            o_ps = {}

            def emit_group(m, g):
                """Group g of block m: s-tiles (2g, 2g+1), q cols of block m."""
                qloc = qloc_of[m]
                n_g = 2 * (m + 1)          # groups in this block
                if m not in o_ps:
                    o_ps[m] = psO.tile([H + 1, QB], f32, tag="o",
                                       name=f"o_{m}")
                s_ps = psS.tile([128, 2 * QB], f32, tag="s",
                                name=f"s_{m}_{g}")
                s0 = 2 * g
                diag = g >= n_g - 2        # pair 0 -> d=(0,1), 1 -> d=(2,3)
                pair = g - (n_g - 2)
                for half in range(2):
                    # diagonal tile d only has valid queries qq >= 128*d:
                    # trim scores (and exp/mask/PV below) to that range
                    q0c = 128 * (2 * pair + half) if diag else 0
                    off = half * QB + q0c
                    nc.tensor.matmul(s_ps[:, off:off + QB - q0c],
                                     kvT_sb[:, (s0 + half) * ST:
                                            (s0 + half + 1) * ST],
                                     qT_sb[:, qloc + q0c:qloc + QB],
                                     start=True, stop=True)
                # drain pipeline at lag 2 (between this group's scores and exp
                # so the PE stream interleaves S,S,PV,PV per group)
                while len(pend) > 2:
                    drain_pv()
                p_sb = pwork.tile([128, 2 * QB], bf16, tag="p",
                                  name=f"p_{m}_{g}")
                if not diag:
                    nc.scalar.activation(p_sb, s_ps,
                                         mybir.ActivationFunctionType.Exp)
                else:
                    for half in range(2):
                        q0c = 128 * (2 * pair + half)
                        off = half * QB + q0c
                        nc.scalar.activation(
                            p_sb[:, off:off + QB - q0c],
                            s_ps[:, off:off + QB - q0c],
                            mybir.ActivationFunctionType.Exp)
                        nc.vector.tensor_mul(
                            p_sb[:, off:off + QB - q0c],
                            p_sb[:, off:off + QB - q0c],
                            mask_sb[:, pair, off:off + QB - q0c])
                pend.append((m, g, p_sb))

            def drain_pv():
                m, g, p_sb = pend.pop(0)
                n_g = 2 * (m + 1)
                o = o_ps[m]
                s0 = 2 * g
                diag = g >= n_g - 2
                pair = g - (n_g - 2)
                for half in range(2):
                    q0c = 128 * (2 * pair + half) if diag else 0
                    off = half * QB + q0c
                    # sub-range start is safe: start=True clears has_written
                    # for the whole bank, so untouched cols overwrite on
                    # their first (later) full-width PV
                    nc.tensor.matmul(o[:, q0c:QB], vp_sb[:, s0 + half, :],
                                     p_sb[:, off:off + QB - q0c],
                                     start=(emitted[m] == 0 and half == 0),
                                     stop=(emitted[m] == n_g - 1
                                           and half == 1))
                emitted[m] += 1
                if emitted[m] == n_g:      # block done -> evacuate + store
                    qloc = qloc_of[m]
                    nc.vector.tensor_copy(oT_sb[:, qloc:qloc + QB], o)
                    nc.scalar.dma_start(out=out_d[:, qloc:qloc + QB],
                                        in_=oT_sb[:, qloc:qloc + QB])

            # ---- chunk loop: project, then flash all ready groups ----
            # group g of block m is ready once chunk m (its queries) and
            # chunk (2g+1)//4 (its keys) have been projected.  Groups may
            # run out of order: PSUM accumulation is order-agnostic (only
            # the first/last PV of a block carry start/stop).
            arrived = set()
            done_g = {m: set() for m in own}

            def emit_ready():
                for m in own:
                    if m not in arrived:
                        continue
                    for g in range(2 * (m + 1)):
                        if g not in done_g[m] and \
                                (2 * g + 1) // 4 in arrived:
                            emit_group(m, g)
                            done_g[m].add(g)

            for c in ch_order:
                t0 = c * TCH
                owned = c in qloc_of
                # drain old PVs here: they keep the PE busy while this
                # chunk's x DMA lands and the DVE evacuates
                while len(pend) > 1:
                    drain_pv()
                kv_ps = psKV.tile([128, TCH], f32, tag="kv", name=f"kv{c}")
                q_ps = (psQT.tile([128, TCH], f32, tag="q", name=f"q{c}")
                        if owned else None)
                for e in range(N_ETILES):
                    nc.tensor.matmul(kv_ps, wkv_sb[:, e, :],
                                     x_sb[:, e, t0:t0 + TCH],
                                     start=(e == 0), stop=(e == N_ETILES - 1))
                if owned:
                    for e in range(N_ETILES):
                        nc.tensor.matmul(q_ps, wq_sb[:, e, :],
                                         x_sb[:, e, t0:t0 + TCH],
                                         start=(e == 0),
                                         stop=(e == N_ETILES - 1))
                nc.vector.tensor_copy(kvT_sb[0:64, t0:t0 + TCH],
                                      kv_ps[0:64, :])
                vT_tmp = pwork.tile([64, TCH], bf16, tag="vt", name=f"vT{c}")
                nc.vector.tensor_copy(vT_tmp, kv_ps[64:128, :])
                if owned:
                    qloc = qloc_of[c]
                    nc.vector.tensor_copy(qT_sb[:, qloc:qloc + QB], q_ps)
                # V^T -> V' via PE transposes, batched into one PSUM tile
                vt_ps = psQT.tile([128, 4, H], bf16, tag="q", name=f"vt{c}")
                for st in range(4):
                    nc.tensor.transpose(vt_ps[:, st, :],
                                        vT_tmp[:, st * ST:(st + 1) * ST],
                                        ident)
                nc.vector.tensor_copy(vp_sb[:, 4 * c:4 * c + 4, 0:H], vt_ps)
                arrived.add(c)
                emit_ready()
            while pend:
                drain_pv()

    nc.finalize()
    return nc


def _tf32(a):
    b = np.ascontiguousarray(a, dtype=np.float32).view(np.uint32)
    r = ((b >> 13) & 1).astype(np.uint32)
    b = (b + 0x0FFF + r) & np.uint32(0xFFFFE000)
    return b.view(np.float32)


def _make_masks():
    ss = np.arange(128)[:, None]
    qq = np.arange(QB)[None, :]
    tiles = [(d * ST + ss <= qq) for d in range(4)]
    m = np.stack([np.concatenate([tiles[0], tiles[1]], axis=1),
                  np.concatenate([tiles[2], tiles[3]], axis=1)])
    return m.astype(ml_dtypes.bfloat16)


_NC_CACHE = {}


def run(x, Wq, Wk, Wv, trace=False):
    wkv = np.concatenate([np.asarray(Wk), np.asarray(Wv)],
                         axis=1).astype(np.float16)
    wq = np.concatenate(
        [np.asarray(Wq), np.zeros((E, 64), np.float32)],
        axis=1).astype(np.float16)
    masks = _make_masks()
    xT = [np.asarray(x)[b].T.astype(np.float16) for b in range(B)]

    outs = [None] * NCORES
    results = []
    for j in (0, 1):
        if j not in _NC_CACHE:
            _NC_CACHE[j] = build_nc(j)
        nc = _NC_CACHE[j]
        cores = [2 * b_ + j for b_ in range(B)]
        in_maps = [{"xT": xT[b_], "wkv": wkv, "wq": wq, "masks": masks}
                   for b_ in range(B)]
        res = run_bass_kernel_spmd(nc, in_maps, core_ids=cores, trace=trace)
        results.append(res)
        for i, c in enumerate(cores):
            outs[c] = res.results[i]["out"]

    full = np.empty((B, T, H), dtype=np.float32)
    inv_sqrt_h = 1.0 / math.sqrt(H)
    for c in range(NCORES):
        b_, j = c // 2, c % 2
        oT = outs[c].astype(np.float32)   # [H+1, 2048] unnormalized bf16
        o = (oT[0:H] / oT[H:H + 1] * inv_sqrt_h).T   # [2048, H]
        for i, m in enumerate(sorted(_blocks(j))):
            full[b_, m * QB:(m + 1) * QB] = o[i * QB:(i + 1) * QB]
    return full, results


def kernel(x, Wq, Wk, Wv):
    out, _ = run(x, Wq, Wk, Wv)
    return out
